# revision 42
# baseline (speedup 1.0000x reference)
# Causal self-attention kernel for 8 Trainium2 NeuronCores.
#
# Problem (hardcoded): B=2, S=2048, D=1024, H=16 heads of dk=64.
#   q,k,v = x @ W.T + b (torch Linear), per-head causal softmax attention,
#   out[b,s,:] = concat_h(attn_h @ v_h). No output projection.
#
# Sharding: 8 cores = 2 batches x 4 head-groups. Core c handles batch c//4
# and heads [4*(c%4), 4*(c%4)+4) => output channels [256*(c%4), +256).
# No cross-device communication.
#
# Per-core design (v2, engine-balance driven):
#   - Everything on the PE runs in bf16 (full 1 cyc/row at any moving size;
#     rel err ~3e-3 vs the 2e-2 gate). PSUM accumulation stays f32.
#   - x arrives as bf16 and is DMA-TRANSPOSED (xbar) straight into
#     xT [d%128, d//128, s] — no PE transposes, no PSUM->SBUF copies.
#   - Projections: qT/kT [e, s] via w as lhsT; q bias folded into the DVE
#     PSUM->SBUF copy; k carries NO bias (per-q score constants are
#     softmax-invariant). v bias lands via a broadcast DVE add; v's 65th
#     column is pure bias 1.0, so PV also produces softmax denominators.
#   - Attention per (head, q-half): key-block j outer; scoresT[sk,sq] on PE
#     ([64,128] lhsT x [64,segw] rhs), one exp per segment on ACT (the
#     pacing engine: ~0.83ns/elem/lane), 0/1 mask multiply on Pool for the
#     diagonal block.
#   - PV produces out[q, e] DIRECTLY: per q-block, lhsT = at[:, qcol:+128]
#     (bf16), rhs = v[k, 65] -> 65-row matmuls into per-q-block PSUM
#     accumulators ([P, 4*65] tiles, 2 per half). Halves PV PE time vs the
#     transposed layout and kills the whole output-transpose tail.
#   - A q-block's accumulator is final at its diagonal j: reciprocal of the
#     ones-column sum + scale to out_sb happen mid-loop, and for the last
#     head the out DMA fires per block.
#   - Projection pieces are drip-fed into the attention phase ("filler") to
#     keep the PE continuously busy (p-state!) while ACT grinds exp.
import numpy as np
import ml_dtypes

B, S, D, H = 2, 2048, 1024, 16
DK = D // H            # 64
NCORES = 8
HPC = 4                # heads per core
E = HPC * DK           # 256 output channels per core
EA = HPC * (DK + 1)    # 260 augmented v width (ones col per head)
P = 128
NSB = S // P           # 16 s-blocks
NDC = D // P           # 8 d-chunks
NEB = E // P           # 2 e-blocks
HALF = 1024
BF = ml_dtypes.bfloat16

_cache = {}


def _build_module():
    import concourse.bacc as bacc
    import concourse.mybir as mybir
    import concourse.tile as tile

    f32 = mybir.dt.float32
    bf16 = mybir.dt.bfloat16
    Exp = mybir.ActivationFunctionType.Exp

    nc = bacc.Bacc("TRN2", target_bir_lowering=False, debug=False)

    x_d = nc.dram_tensor("x", [S, D], bf16, kind="ExternalInput")
    wq_d = nc.dram_tensor("wq_t", [D, E], bf16, kind="ExternalInput")
    wk_d = nc.dram_tensor("wk_t", [D, E], bf16, kind="ExternalInput")
    wv_d = nc.dram_tensor("wv_t", [D, EA], bf16, kind="ExternalInput")
    bq_d = nc.dram_tensor("bq", [1, E], f32, kind="ExternalInput")
    bv_d = nc.dram_tensor("bv", [P, EA], bf16, kind="ExternalInput")
    mask_d = nc.dram_tensor("mask", [P, P], bf16, kind="ExternalInput")
    out_d = nc.dram_tensor("out", [S, E], f32, kind="ExternalOutput")

    with tile.TileContext(nc) as tc:
        with (
            tc.tile_pool(name="consts", bufs=1) as consts,
            tc.tile_pool(name="qkv", bufs=1) as qkv,
            tc.tile_pool(name="outst", bufs=1) as outst,
            tc.tile_pool(name="xt", bufs=1) as xtp,
            tc.tile_pool(name="pp", bufs=2, space="PSUM") as pp,
            tc.tile_pool(name="psc", bufs=2, space="PSUM") as pscp,
            tc.tile_pool(name="pacc", bufs=2, space="PSUM") as paccp,
            tc.tile_pool(name="attn", bufs=5) as attnp,
            tc.tile_pool(name="lin", bufs=4) as linp,
        ):
            # ---- constants / staging tiles ----
            wq_sb = consts.tile([P, NDC, E], bf16, tag="wq")
            wk_sb = consts.tile([P, NDC, E], bf16, tag="wk")
            wv_sb = consts.tile([P, NDC, EA], bf16, tag="wv")
            bqc_sb = consts.tile([P, NEB], f32, tag="bqc")
            bv_sb = consts.tile([P, EA], bf16, tag="bv")
            mask_sb = consts.tile([P, P], bf16, tag="mask")

            xT = xtp.tile([P, NDC, S], bf16, tag="xT")
            qT = qkv.tile([P, NEB, S], bf16, tag="qT")
            kT = qkv.tile([P, NEB, S], bf16, tag="kT")
            v_sb = qkv.tile([P, NSB, EA], bf16, tag="v")
            out_sb = outst.tile([P, NSB, E], f32, tag="out")

            # ---- DMAs: x transposes on the SP queue, weights on ACT's ----
            XCH = 256
            NXCH = S // XCH

            def emit_xch(i):
                nc.sync.dma_start_transpose(
                    out=xT[:, :, i * XCH:(i + 1) * XCH],
                    in_=x_d[i * XCH:(i + 1) * XCH, :],
                )

            def emit_wv_slice(h):
                c0 = h * 65
                nc.sync.dma_start(
                    out=wv_sb[:, :, c0:c0 + 65],
                    in_=wv_d[:, c0:c0 + 65].rearrange("(c p) e -> p c e", p=P))

            def emit_w_half(w_sb, w_d, eb):
                nc.sync.dma_start(
                    out=w_sb[:, :, eb * P:(eb + 1) * P],
                    in_=w_d[:, eb * P:(eb + 1) * P].rearrange(
                        "(c p) e -> p c e", p=P))

            nc.sync.dma_start(
                out=wq_sb, in_=wq_d[:].rearrange("(c p) e -> p c e", p=P))
            nc.sync.dma_start(
                out=bqc_sb, in_=bq_d[:].rearrange("o (c p) -> p (o c)", p=P))
            emit_xch(0)
            nc.sync.dma_start(
                out=wk_sb, in_=wk_d[:].rearrange("(c p) e -> p c e", p=P))
            emit_xch(1)
            nc.sync.dma_start(out=mask_sb, in_=mask_d[:])
            emit_wv_slice(0)
            nc.sync.dma_start(out=bv_sb, in_=bv_d[:])
            emit_xch(2)
            emit_wv_slice(1)
            emit_xch(3)
            emit_wv_slice(2)
            emit_xch(4)
            emit_wv_slice(3)
            for i in range(5, NXCH):
                emit_xch(i)

            # ---- projection pieces ----
            def emit_qk_proj(which, eb, lo, w):
                # qT/kT columns [lo, lo+w) for e-block eb. k carries no
                # bias: scores' per-q constant q~.bk is softmax-invariant.
                w_sb = wq_sb if which == 0 else wk_sb
                dst = qT if which == 0 else kT
                ps = pp.tile([P, 512], f32, tag="pp")
                for dc in range(NDC):
                    nc.tensor.matmul(
                        ps[:, :w],
                        lhsT=w_sb[:, dc, eb * P:(eb + 1) * P],
                        rhs=xT[:, dc, lo:lo + w],
                        start=(dc == 0),
                        stop=(dc == NDC - 1),
                    )
                if which == 0:
                    nc.vector.tensor_scalar_add(
                        dst[:, eb, lo:lo + w], ps[:, :w], bqc_sb[:, eb:eb + 1])
                else:
                    nc.vector.tensor_copy(dst[:, eb, lo:lo + w], ps[:, :w])

            vdone = set()

            def emit_v_proj(sb, h):
                # per-(s-block, head) piece: lands the PE work exactly in
                # the phase that consumes it
                if (sb, h) in vdone:
                    return
                vdone.add((sb, h))
                c0 = h * 65
                ps = pp.tile([P, 512], f32, tag="pp")
                pv = ps[:, :65]
                for dc in range(NDC):
                    nc.tensor.matmul(
                        pv,
                        lhsT=xT[:, dc, sb * P:(sb + 1) * P],
                        rhs=wv_sb[:, dc, c0:c0 + 65],
                        start=(dc == 0),
                        stop=(dc == NDC - 1),
                    )
                nc.vector.tensor_add(
                    v_sb[:, sb, c0:c0 + 65], pv, bv_sb[:, c0:c0 + 65])

            # ---- attention over query range [lo, hi) for one head ----
            def attn_head_span(h, lo, hi, drips={}, dma_out=False,
                               j0_split=None, j0_mid_hook=None):
                po = DK * (h % 2)
                eb = h // 2
                kT_h = kT[po:po + DK, eb, :]
                qT_h = qT[po:po + DK, eb, :]
                nj = hi // P
                qa0 = lo // P
                n_chains = nj - qa0
                accs = []
                for _ in range((n_chains + 3) // 4):
                    acc_t = paccp.tile([P, 4 * 65], f32, tag="acc")
                    accs.append(acc_t)

                def emit_norm(qa, acc, col):
                    linv = linp.tile([P, 1], f32, tag="linv")
                    nc.vector.reciprocal(linv, acc[:, col + DK:col + DK + 1])
                    nc.vector.tensor_scalar_mul(
                        out_sb[:, qa, h * DK:(h + 1) * DK],
                        acc[:, col:col + DK], linv)
                    if dma_out:
                        nc.sync.dma_start(
                            out=out_d[qa * P:(qa + 1) * P, :],
                            in_=out_sb[:, qa, :])

                def emit_pv(j, at, sb0):
                    emit_v_proj(j, h)  # self-heal; no-op if emitted earlier
                    rhs_v = v_sb[:, j, h * 65:(h + 1) * 65]
                    qa_lo = max(j, qa0)
                    # diagonal chain (qa == j) last: its lhsT waits the
                    # Pool mask multiply, the others only the exp. (At j=0
                    # keep ascending order — bank start flags lead there.)
                    if j == qa_lo and j > 0 and qa_lo + 1 < nj:
                        order = list(range(qa_lo + 1, nj)) + [qa_lo]
                    else:
                        order = list(range(qa_lo, nj))
                    for qa in order:
                        rel = qa - qa0
                        acc = accs[rel // 4]
                        col = (rel % 4) * 65
                        qc = qa * P - sb0
                        bank_last = min((rel // 4) * 4 + 3, n_chains - 1)
                        # One accumulation group per PSUM bank: the first
                        # matmul into the bank zeroes the whole 2KB region
                        # (start), the bank's last chain closes it (stop).
                        nc.tensor.matmul(
                            acc[:, col:col + 65],
                            lhsT=at[:, qc:qc + P],
                            rhs=rhs_v,
                            start=(j == 0 and rel % 4 == 0),
                            stop=(j == qa and rel == bank_last),
                        )
                        if j == qa and rel == bank_last:
                            # bank group just closed: normalize its chains
                            for q2 in range(qa - (rel % 4), qa + 1):
                                emit_norm(q2, acc, ((q2 - qa0) % 4) * 65)

                pending = None
                for j in range(nj):
                    ko = j * P
                    sb0 = max(ko, lo)
                    segw = hi - sb0
                    ps = pscp.tile([P, HALF], f32, tag="sc")
                    lhsT_k = kT_h[:, ko:ko + P]
                    at = attnp.tile([P, HALF], bf16, tag="at")
                    if j == 0 and j0_split:
                        pieces = [(0, j0_split), (j0_split, segw)]
                    else:
                        pieces = [(0, segw)]
                    for pi, (a, b) in enumerate(pieces):
                        m = a
                        while m < b:
                            w = min(512 - m % 512, b - m)
                            nc.tensor.matmul(
                                ps[:, m:m + w],
                                lhsT=lhsT_k,
                                rhs=qT_h[:, sb0 + m:sb0 + m + w],
                                start=True,
                                stop=True,
                            )
                            m += w
                        nc.scalar.activation(
                            out=at[:, a:b], in_=ps[:, a:b], func=Exp,
                            scale=0.125)
                        if pi == 0 and j == 0 and j0_mid_hook is not None:
                            j0_mid_hook()
                    if ko >= lo:
                        nc.gpsimd.tensor_mul(at[:, 0:P], at[:, 0:P], mask_sb)
                    if j in drips:
                        drips[j]()
                    if pending is not None:
                        emit_pv(*pending)
                    pending = (j, at, sb0)
                emit_pv(*pending)

            # ---- schedule ----
            # warm the ACT exp table off the critical path
            warm_in = linp.tile([1, 1], f32, tag="warm_in")
            warm_out = linp.tile([1, 1], f32, tag="warm_out")
            nc.vector.memset(warm_in, 0.0)
            nc.scalar.activation(out=warm_out, in_=warm_in, func=Exp)

            # PE p-state warm-up: dummy matmuls while the first DMAs land,
            # so the real projections start at full clock
            dum = linp.tile([P, 512], bf16, tag="dum")
            nc.vector.memset(dum, 0.0)

            def emit_dummies(n):
                for _ in range(n):
                    pd = pp.tile([P, 512], f32, tag="pp")
                    nc.tensor.matmul(
                        pd, lhsT=dum[:, :P], rhs=dum, start=True, stop=True)

            def qk(w, e, lo, wd):
                return lambda: emit_qk_proj(w, e, lo, wd)

            # Span schedule: spans are causally independent, so order them
            # to (a) match x-chunk DMA arrival at the start, (b) keep the
            # heavy-ACT [1024,2048) phases mid-stream where projection
            # drips keep the PE saturated, (c) end on light quarter spans
            # whose ACT and PE loads balance.
            emit_dummies(11)
            emit_qk_proj(0, 0, 0, 256)      # q eb0 cols 0-255    (x0)
            emit_dummies(3)
            emit_qk_proj(1, 0, 0, 256)      # k eb0 cols 0-255    (wk)

            def p1_hook():
                emit_dummies(5)                 # bridge to x1 landing
                emit_qk_proj(0, 0, 256, 256)    # x1
                emit_qk_proj(1, 0, 256, 256)

            attn_head_span(0, 0, 512, j0_split=256, j0_mid_hook=p1_hook)
            attn_head_span(1, 0, 512, drips={1: qk(0, 0, 512, 256),
                                             3: qk(1, 0, 512, 256)})

            def p3_hook():
                emit_qk_proj(0, 0, 768, 256)    # x3
                emit_qk_proj(1, 0, 768, 256)

            attn_head_span(0, 512, 1024, j0_split=256, j0_mid_hook=p3_hook,
                           drips={1: qk(0, 1, 0, 256),
                                  3: qk(0, 1, 256, 256)})
            attn_head_span(1, 512, 1024, drips={1: qk(1, 1, 0, 256),
                                                3: qk(1, 1, 256, 256)})
            attn_head_span(2, 0, 512, drips={1: qk(0, 0, 1024, 512)})
            attn_head_span(3, 0, 512, drips={1: qk(0, 0, 1536, 512)},
                           dma_out=True)
            attn_head_span(0, 1024, 2048, drips={1: qk(1, 0, 1024, 512),
                                                 4: qk(1, 0, 1536, 512)})
            attn_head_span(1, 1024, 2048, drips={1: qk(0, 1, 1024, 512),
                                                 4: qk(0, 1, 1536, 512)})
            attn_head_span(2, 1024, 2048, drips={1: qk(1, 1, 512, 512),
                                                 4: qk(1, 1, 1024, 512),
                                                 7: qk(1, 1, 1536, 512)})
            attn_head_span(3, 1024, 2048, drips={1: qk(0, 1, 512, 512)},
                           dma_out=True)
            attn_head_span(2, 512, 1024)
            attn_head_span(3, 512, 1024, dma_out=True)

    nc.compile()
    return nc


def _prep_core_inputs(inputs, c):
    b, hg = c // HPC, c % HPC
    e0 = hg * E

    x = np.asarray(inputs["x"], dtype=np.float32)
    wq = np.asarray(inputs["Wq"], dtype=np.float32)
    wk = np.asarray(inputs["Wk"], dtype=np.float32)
    wv = np.asarray(inputs["Wv"], dtype=np.float32)
    bq = np.asarray(inputs["bq"], dtype=np.float32)
    bk = np.asarray(inputs["bk"], dtype=np.float32)
    bv = np.asarray(inputs["bv"], dtype=np.float32)

    wq_t = np.ascontiguousarray(wq[e0:e0 + E, :].T).astype(BF)   # [D, E]
    wk_t = np.ascontiguousarray(wk[e0:e0 + E, :].T).astype(BF)
    wv_t = np.zeros((D, EA), dtype=np.float32)
    bv_a = np.zeros((1, EA), dtype=np.float32)
    for lh in range(HPC):
        cols = slice(lh * 65, lh * 65 + DK)
        rows = slice(e0 + lh * DK, e0 + lh * DK + DK)
        wv_t[:, cols] = wv[rows, :].T
        bv_a[0, cols] = bv[rows]
        bv_a[0, lh * 65 + DK] = 1.0                              # ones column

    mask = np.where(
        np.arange(P)[None, :] >= np.arange(P)[:, None], 1.0, 0.0
    ).astype(BF)

    return {
        "x": np.ascontiguousarray(x[b]).astype(BF),
        "wq_t": wq_t,
        "wk_t": wk_t,
        "wv_t": wv_t.astype(BF),
        "bq": np.ascontiguousarray(bq[e0:e0 + E])[None, :],
        "bv": np.tile(bv_a.astype(BF), (P, 1)),
        "mask": mask,
    }


def kernel(**inputs):
    from concourse.bass_utils import run_bass_kernel_spmd

    if "nc" not in _cache:
        _cache["nc"] = _build_module()
    nc = _cache["nc"]

    in_maps = [_prep_core_inputs(inputs, c) for c in range(NCORES)]
    res = run_bass_kernel_spmd(nc, in_maps, core_ids=list(range(NCORES)))

    out = np.empty((B, S, D), dtype=np.float32)
    for c in range(NCORES):
        b, hg = c // HPC, c % HPC
        out[b, :, hg * E:(hg + 1) * E] = res.results[c]["out"]
    return out


# revision 48
# speedup vs baseline: 1.3832x; 1.3832x over previous
# Causal self-attention kernel for 8 Trainium2 NeuronCores.
#
# Problem (hardcoded): B=2, S=2048, D=1024, H=16 heads of dk=64.
#   q,k,v = x @ W.T + b (torch Linear), per-head causal softmax attention,
#   out[b,s,:] = concat_h(attn_h @ v_h). No output projection.
#
# Sharding: 8 cores = 2 batches x 4 head-groups. Core c handles batch c//4
# and heads [4*(c%4), 4*(c%4)+4) => output channels [256*(c%4), +256).
# No cross-device communication.
#
# Per-core design (v2, engine-balance driven):
#   - Everything on the PE runs in bf16 (full 1 cyc/row at any moving size;
#     rel err ~3e-3 vs the 2e-2 gate). PSUM accumulation stays f32.
#   - x arrives as bf16 and is DMA-TRANSPOSED (xbar) straight into
#     xT [d%128, d//128, s] — no PE transposes, no PSUM->SBUF copies.
#   - Projections: qT/kT [e, s] via w as lhsT; q bias folded into the DVE
#     PSUM->SBUF copy; k carries NO bias (per-q score constants are
#     softmax-invariant). v bias lands via a broadcast DVE add; v's 65th
#     column is pure bias 1.0, so PV also produces softmax denominators.
#   - Attention per (head, q-half): key-block j outer; scoresT[sk,sq] on PE
#     ([64,128] lhsT x [64,segw] rhs), one exp per segment on ACT (the
#     pacing engine: ~0.83ns/elem/lane), 0/1 mask multiply on Pool for the
#     diagonal block.
#   - PV produces out[q, e] DIRECTLY: per q-block, lhsT = at[:, qcol:+128]
#     (bf16), rhs = v[k, 65] -> 65-row matmuls into per-q-block PSUM
#     accumulators ([P, 4*65] tiles, 2 per half). Halves PV PE time vs the
#     transposed layout and kills the whole output-transpose tail.
#   - A q-block's accumulator is final at its diagonal j: reciprocal of the
#     ones-column sum + scale to out_sb happen mid-loop, and for the last
#     head the out DMA fires per block.
#   - Projection pieces are drip-fed into the attention phase ("filler") to
#     keep the PE continuously busy (p-state!) while ACT grinds exp.
import numpy as np
import ml_dtypes

B, S, D, H = 2, 2048, 1024, 16
DK = D // H            # 64
NCORES = 8
HPC = 4                # heads per core
E = HPC * DK           # 256 output channels per core
EA = HPC * (DK + 1)    # 260 augmented v width (ones col per head)
P = 128
NSB = S // P           # 16 s-blocks
NDC = D // P           # 8 d-chunks
NEB = E // P           # 2 e-blocks
HALF = 1024
BF = ml_dtypes.bfloat16

_cache = {}


def _build_module():
    import concourse.bacc as bacc
    import concourse.mybir as mybir
    import concourse.tile as tile

    f32 = mybir.dt.float32
    bf16 = mybir.dt.bfloat16
    Exp = mybir.ActivationFunctionType.Exp

    nc = bacc.Bacc("TRN2", target_bir_lowering=False, debug=False)

    x_d = nc.dram_tensor("x", [S, D], bf16, kind="ExternalInput")
    wq_d = nc.dram_tensor("wq_t", [D, E], bf16, kind="ExternalInput")
    wk_d = nc.dram_tensor("wk_t", [D, E], bf16, kind="ExternalInput")
    wv_d = nc.dram_tensor("wv_t", [D, EA], bf16, kind="ExternalInput")
    bq_d = nc.dram_tensor("bq", [1, E], f32, kind="ExternalInput")
    bv_d = nc.dram_tensor("bv", [P, EA], bf16, kind="ExternalInput")
    mask_d = nc.dram_tensor("mask", [P, P], bf16, kind="ExternalInput")
    out_d = nc.dram_tensor("out", [S, E], f32, kind="ExternalOutput")

    with tile.TileContext(nc) as tc:
        with (
            tc.tile_pool(name="consts", bufs=1) as consts,
            tc.tile_pool(name="qkv", bufs=1) as qkv,
            tc.tile_pool(name="outst", bufs=1) as outst,
            tc.tile_pool(name="xt", bufs=1) as xtp,
            tc.tile_pool(name="pp", bufs=2, space="PSUM") as pp,
            tc.tile_pool(name="psc", bufs=2, space="PSUM") as pscp,
            tc.tile_pool(name="pacc", bufs=2, space="PSUM") as paccp,
            tc.tile_pool(name="attn", bufs=5) as attnp,
            tc.tile_pool(name="lin", bufs=4) as linp,
        ):
            # ---- constants / staging tiles ----
            wq_sb = consts.tile([P, NDC, E], bf16, tag="wq")
            wk_sb = consts.tile([P, NDC, E], bf16, tag="wk")
            wv_sb = consts.tile([P, NDC, EA], bf16, tag="wv")
            bqc_sb = consts.tile([P, NEB], f32, tag="bqc")
            bv_sb = consts.tile([P, EA], bf16, tag="bv")
            mask_sb = consts.tile([P, P], bf16, tag="mask")

            xT = xtp.tile([P, NDC, S], bf16, tag="xT")
            qT = qkv.tile([P, NEB, S], bf16, tag="qT")
            kT = qkv.tile([P, NEB, S], bf16, tag="kT")
            v_sb = qkv.tile([P, NSB, EA], bf16, tag="v")
            out_sb = outst.tile([P, NSB, E], f32, tag="out")

            # ---- DMAs: x transposes on the SP queue, weights on ACT's ----
            XCH = 256
            NXCH = S // XCH

            def emit_xch(i):
                nc.sync.dma_start_transpose(
                    out=xT[:, :, i * XCH:(i + 1) * XCH],
                    in_=x_d[i * XCH:(i + 1) * XCH, :],
                )

            def emit_wv_slice(h):
                c0 = h * 65
                nc.sync.dma_start(
                    out=wv_sb[:, :, c0:c0 + 65],
                    in_=wv_d[:, c0:c0 + 65].rearrange("(c p) e -> p c e", p=P))

            def emit_w_half(w_sb, w_d, eb):
                nc.sync.dma_start(
                    out=w_sb[:, :, eb * P:(eb + 1) * P],
                    in_=w_d[:, eb * P:(eb + 1) * P].rearrange(
                        "(c p) e -> p c e", p=P))

            nc.sync.dma_start(
                out=wq_sb, in_=wq_d[:].rearrange("(c p) e -> p c e", p=P))
            nc.sync.dma_start(
                out=bqc_sb, in_=bq_d[:].rearrange("o (c p) -> p (o c)", p=P))
            emit_xch(0)
            nc.sync.dma_start(
                out=wk_sb, in_=wk_d[:].rearrange("(c p) e -> p c e", p=P))
            emit_xch(1)
            nc.sync.dma_start(out=mask_sb, in_=mask_d[:])
            emit_wv_slice(0)
            nc.sync.dma_start(out=bv_sb, in_=bv_d[:])
            emit_xch(2)
            emit_wv_slice(1)
            emit_xch(3)
            emit_wv_slice(2)
            emit_xch(4)
            emit_wv_slice(3)
            for i in range(5, NXCH):
                emit_xch(i)

            # ---- projection pieces ----
            def emit_qk_proj(which, eb, lo, w):
                # qT/kT columns [lo, lo+w) for e-block eb. k carries no
                # bias: scores' per-q constant q~.bk is softmax-invariant.
                w_sb = wq_sb if which == 0 else wk_sb
                dst = qT if which == 0 else kT
                ps = pp.tile([P, 512], f32, tag="pp")
                for dc in range(NDC):
                    nc.tensor.matmul(
                        ps[:, :w],
                        lhsT=w_sb[:, dc, eb * P:(eb + 1) * P],
                        rhs=xT[:, dc, lo:lo + w],
                        start=(dc == 0),
                        stop=(dc == NDC - 1),
                    )
                if which == 0:
                    nc.vector.tensor_scalar_add(
                        dst[:, eb, lo:lo + w], ps[:, :w], bqc_sb[:, eb:eb + 1])
                else:
                    nc.vector.tensor_copy(dst[:, eb, lo:lo + w], ps[:, :w])

            vdone = set()

            def emit_v_proj(sb, h):
                # per-(s-block, head) piece: lands the PE work exactly in
                # the phase that consumes it
                if (sb, h) in vdone:
                    return
                vdone.add((sb, h))
                c0 = h * 65
                ps = pp.tile([P, 512], f32, tag="pp")
                pv = ps[:, :65]
                for dc in range(NDC):
                    nc.tensor.matmul(
                        pv,
                        lhsT=xT[:, dc, sb * P:(sb + 1) * P],
                        rhs=wv_sb[:, dc, c0:c0 + 65],
                        start=(dc == 0),
                        stop=(dc == NDC - 1),
                    )
                nc.vector.tensor_add(
                    v_sb[:, sb, c0:c0 + 65], pv, bv_sb[:, c0:c0 + 65])

            # ---- attention over query range [lo, hi) for one head ----
            def attn_head_span(h, lo, hi, drips={}, dma_out=False,
                               j0_split=None, j0_mid_hook=None):
                po = DK * (h % 2)
                eb = h // 2
                kT_h = kT[po:po + DK, eb, :]
                qT_h = qT[po:po + DK, eb, :]
                nj = hi // P
                qa0 = lo // P
                n_chains = nj - qa0
                accs = []
                for _ in range((n_chains + 3) // 4):
                    acc_t = paccp.tile([P, 4 * 65], f32, tag="acc")
                    accs.append(acc_t)

                def emit_norm(qa, acc, col):
                    linv = linp.tile([P, 1], f32, tag="linv")
                    nc.vector.reciprocal(linv, acc[:, col + DK:col + DK + 1])
                    nc.vector.tensor_scalar_mul(
                        out_sb[:, qa, h * DK:(h + 1) * DK],
                        acc[:, col:col + DK], linv)
                    if dma_out:
                        nc.sync.dma_start(
                            out=out_d[qa * P:(qa + 1) * P, :],
                            in_=out_sb[:, qa, :])

                def emit_pv(j, at, sb0):
                    emit_v_proj(j, h)  # self-heal; no-op if emitted earlier
                    rhs_v = v_sb[:, j, h * 65:(h + 1) * 65]
                    qa_lo = max(j, qa0)
                    # diagonal chain (qa == j) last: its lhsT waits the
                    # Pool mask multiply, the others only the exp. (At j=0
                    # keep ascending order — bank start flags lead there.)
                    if j == qa_lo and j > 0 and qa_lo + 1 < nj:
                        order = list(range(qa_lo + 1, nj)) + [qa_lo]
                    else:
                        order = list(range(qa_lo, nj))
                    for qa in order:
                        rel = qa - qa0
                        acc = accs[rel // 4]
                        col = (rel % 4) * 65
                        qc = qa * P - sb0
                        bank_last = min((rel // 4) * 4 + 3, n_chains - 1)
                        # One accumulation group per PSUM bank: the first
                        # matmul into the bank zeroes the whole 2KB region
                        # (start), the bank's last chain closes it (stop).
                        nc.tensor.matmul(
                            acc[:, col:col + 65],
                            lhsT=at[:, qc:qc + P],
                            rhs=rhs_v,
                            start=(j == 0 and rel % 4 == 0),
                            stop=(j == qa and rel == bank_last),
                        )
                        if j == qa and rel == bank_last:
                            # bank group just closed: normalize its chains
                            for q2 in range(qa - (rel % 4), qa + 1):
                                emit_norm(q2, acc, ((q2 - qa0) % 4) * 65)

                pending = None
                for j in range(nj):
                    ko = j * P
                    sb0 = max(ko, lo)
                    segw = hi - sb0
                    ps = pscp.tile([P, HALF], f32, tag="sc")
                    lhsT_k = kT_h[:, ko:ko + P]
                    at = attnp.tile([P, HALF], bf16, tag="at")
                    if j == 0 and j0_split:
                        pieces = [(0, j0_split), (j0_split, segw)]
                    else:
                        pieces = [(0, segw)]
                    for pi, (a, b) in enumerate(pieces):
                        m = a
                        while m < b:
                            w = min(512 - m % 512, b - m)
                            nc.tensor.matmul(
                                ps[:, m:m + w],
                                lhsT=lhsT_k,
                                rhs=qT_h[:, sb0 + m:sb0 + m + w],
                                start=True,
                                stop=True,
                            )
                            m += w
                        nc.scalar.activation(
                            out=at[:, a:b], in_=ps[:, a:b], func=Exp,
                            scale=0.125)
                        if pi == 0 and j == 0 and j0_mid_hook is not None:
                            j0_mid_hook()
                    if ko >= lo:
                        nc.gpsimd.tensor_mul(at[:, 0:P], at[:, 0:P], mask_sb)
                    if j in drips:
                        drips[j]()
                    if pending is not None:
                        emit_pv(*pending)
                    pending = (j, at, sb0)
                emit_pv(*pending)

            # ---- schedule ----
            # warm the ACT exp table off the critical path
            warm_in = linp.tile([1, 1], f32, tag="warm_in")
            warm_out = linp.tile([1, 1], f32, tag="warm_out")
            nc.vector.memset(warm_in, 0.0)
            nc.scalar.activation(out=warm_out, in_=warm_in, func=Exp)

            # PE p-state warm-up: dummy matmuls while the first DMAs land,
            # so the real projections start at full clock
            dum = linp.tile([P, 512], bf16, tag="dum")
            nc.vector.memset(dum, 0.0)

            def emit_dummies(n):
                for _ in range(n):
                    pd = pp.tile([P, 512], f32, tag="pp")
                    nc.tensor.matmul(
                        pd, lhsT=dum[:, :P], rhs=dum, start=True, stop=True)

            def qk(w, e, lo, wd):
                return lambda: emit_qk_proj(w, e, lo, wd)

            # Span schedule: spans are causally independent, so order them
            # to (a) match x-chunk DMA arrival at the start, (b) keep the
            # heavy-ACT [1024,2048) phases mid-stream where projection
            # drips keep the PE saturated, (c) end on light quarter spans
            # whose ACT and PE loads balance.
            emit_dummies(10)
            emit_qk_proj(0, 0, 0, 256)      # q eb0 cols 0-255    (x0)
            emit_dummies(3)
            emit_qk_proj(1, 0, 0, 256)      # k eb0 cols 0-255    (wk)

            def p1_hook():
                emit_dummies(3)                 # bridge to x1 landing
                emit_qk_proj(0, 0, 256, 256)    # x1
                emit_qk_proj(1, 0, 256, 256)

            attn_head_span(0, 0, 512, j0_split=256, j0_mid_hook=p1_hook)
            attn_head_span(1, 0, 512, drips={1: qk(0, 0, 512, 256),
                                             3: qk(1, 0, 512, 256)})

            def p3_hook():
                emit_qk_proj(0, 0, 768, 256)    # x3
                emit_qk_proj(1, 0, 768, 256)

            attn_head_span(0, 512, 1024, j0_split=256, j0_mid_hook=p3_hook,
                           drips={1: qk(0, 1, 0, 256),
                                  3: qk(0, 1, 256, 256)})
            attn_head_span(1, 512, 1024, drips={1: qk(1, 1, 0, 256),
                                                3: qk(1, 1, 256, 256)})
            attn_head_span(2, 0, 512, drips={1: qk(0, 0, 1024, 512)})
            attn_head_span(3, 0, 512, drips={1: qk(0, 0, 1536, 512)},
                           dma_out=True)
            attn_head_span(0, 1024, 2048, drips={1: qk(1, 0, 1024, 512),
                                                 4: qk(1, 0, 1536, 512)})
            attn_head_span(1, 1024, 2048, drips={1: qk(0, 1, 1024, 512),
                                                 4: qk(0, 1, 1536, 512)})
            attn_head_span(2, 1024, 2048, drips={1: qk(1, 1, 512, 512),
                                                 4: qk(1, 1, 1024, 512),
                                                 7: qk(1, 1, 1536, 512)})
            attn_head_span(3, 1024, 2048, drips={1: qk(0, 1, 512, 512)},
                           dma_out=True)
            attn_head_span(2, 512, 1024)
            attn_head_span(3, 512, 1024, dma_out=True)

    nc.compile()
    return nc


def _prep_core_inputs(inputs, c):
    b, hg = c // HPC, c % HPC
    e0 = hg * E

    x = np.asarray(inputs["x"], dtype=np.float32)
    wq = np.asarray(inputs["Wq"], dtype=np.float32)
    wk = np.asarray(inputs["Wk"], dtype=np.float32)
    wv = np.asarray(inputs["Wv"], dtype=np.float32)
    bq = np.asarray(inputs["bq"], dtype=np.float32)
    bk = np.asarray(inputs["bk"], dtype=np.float32)
    bv = np.asarray(inputs["bv"], dtype=np.float32)

    wq_t = np.ascontiguousarray(wq[e0:e0 + E, :].T).astype(BF)   # [D, E]
    wk_t = np.ascontiguousarray(wk[e0:e0 + E, :].T).astype(BF)
    wv_t = np.zeros((D, EA), dtype=np.float32)
    bv_a = np.zeros((1, EA), dtype=np.float32)
    for lh in range(HPC):
        cols = slice(lh * 65, lh * 65 + DK)
        rows = slice(e0 + lh * DK, e0 + lh * DK + DK)
        wv_t[:, cols] = wv[rows, :].T
        bv_a[0, cols] = bv[rows]
        bv_a[0, lh * 65 + DK] = 1.0                              # ones column

    mask = np.where(
        np.arange(P)[None, :] >= np.arange(P)[:, None], 1.0, 0.0
    ).astype(BF)

    return {
        "x": np.ascontiguousarray(x[b]).astype(BF),
        "wq_t": wq_t,
        "wk_t": wk_t,
        "wv_t": wv_t.astype(BF),
        "bq": np.ascontiguousarray(bq[e0:e0 + E])[None, :],
        "bv": np.tile(bv_a.astype(BF), (P, 1)),
        "mask": mask,
    }


def kernel(**inputs):
    from concourse.bass_utils import run_bass_kernel_spmd

    if "nc" not in _cache:
        _cache["nc"] = _build_module()
    nc = _cache["nc"]

    in_maps = [_prep_core_inputs(inputs, c) for c in range(NCORES)]
    res = run_bass_kernel_spmd(nc, in_maps, core_ids=list(range(NCORES)))

    out = np.empty((B, S, D), dtype=np.float32)
    for c in range(NCORES):
        b, hg = c // HPC, c % HPC
        out[b, :, hg * E:(hg + 1) * E] = res.results[c]["out"]
    return out
